# revision 4
# baseline (speedup 1.0000x reference)
"""Trainium2 Bass kernel for nn_DiagRNN (diagonal complex linear RNN / LRU).

  y = Re[C @ h] + D*x,  h_t = A h_{t-1} + B x_t  (A complex-diagonal)

Strategy (8 NeuronCores, sequence-parallel):
  * Sequence of L=16384 split into 32 chunks of T=512. Chunk m is processed
    by core m%8 in "slot" m//8 (interleaved assignment) so cross-core carry
    exchange is a small per-slot AllGather that pipelines behind compute.
  * Complex scan is reduced to two REAL first-order scans per chunk via a
    rotating-frame transform: with A = r*e^{i\theta},
        W_k = e^{-i\theta k} h_{mT+k}  satisfies  W_k = r W_{k-1} + g_k,
        g_k = e^{-i\theta k} (B x)_{mT+k}.
    The real scans run on the DVE hardware scan instruction
    (tensor_tensor_scan).  Pre/post rotations are elementwise with
    host-precomputed cos/sin/r^k tables.
  * Carries: cores publish standalone chunk sums E_m, AllGather them, and
    each core folds same-slot predecessors in with one fused
    scalar_tensor_tensor per (slice, re/im); cross-slot history enters for
    free through the scan initial value.
  * Matmuls (B_re, B_im, C projections) run on the PE at 1 cycle/row using
    fp16 operands (B/x) and fp16 C/u; accumulation is fp32 in PSUM.
"""
import sys, os
sys.path.insert(0, '/opt/trn_rl_repo')
import numpy as np

import concourse.bass as bass
import concourse.bacc as bacc
import concourse.tile as tile
import concourse.mybir as mybir
from concourse.bass_utils import run_bass_kernel_spmd

L, H, M = 16384, 1024, 1024
NC = 8
T = 512
S = L // (T * NC)          # 4 slots
NSL = H // 128             # 8 slices

f32 = mybir.dt.float32
f32r = mybir.dt.float32r
f16 = mybir.dt.float16
AL = mybir.AluOpType
AX = mybir.AxisListType

TABLE_DT = f16   # cos/sin/rpow tables
MM_DT = f16      # B, x, C, u matmul operand dtype

_BUILD_CACHE = {}


def _build():
    if "nc" in _BUILD_CACHE:
        return _BUILD_CACHE["nc"]
    nc = bacc.Bacc("TRN2", target_bir_lowering=False, debug=False,
                   num_devices=NC)

    xt_d = nc.dram_tensor("xt", [S, M, T], MM_DT, kind="ExternalInput").ap()
    brt_d = nc.dram_tensor("brt", [M, H], MM_DT, kind="ExternalInput").ap()
    bit_d = nc.dram_tensor("bit", [M, H], MM_DT, kind="ExternalInput").ap()
    ct_d = nc.dram_tensor("ct", [H, M], MM_DT, kind="ExternalInput").ap()
    cos_d = nc.dram_tensor("costb", [H, T], TABLE_DT, kind="ExternalInput").ap()
    sin_d = nc.dram_tensor("sintb", [H, T], TABLE_DT, kind="ExternalInput").ap()
    rpow_d = nc.dram_tensor("rpowtb", [H, T], TABLE_DT, kind="ExternalInput").ap()
    consts_d = nc.dram_tensor("consts", [128, 128], f32, kind="ExternalInput").ap()
    cwfre_d = nc.dram_tensor("cwfre", [128, 128], f32, kind="ExternalInput").ap()
    cwfim_d = nc.dram_tensor("cwfim", [128, 128], f32, kind="ExternalInput").ap()
    rwfre_d = nc.dram_tensor("rwfre", [128, 128], f32, kind="ExternalInput").ap()
    rwfim_d = nc.dram_tensor("rwfim", [128, 128], f32, kind="ExternalInput").ap()
    ident_d = nc.dram_tensor("ident", [128, 128], f32, kind="ExternalInput").ap()
    y_d = nc.dram_tensor("y", [S, M, T], f32, kind="ExternalOutput").ap()

    with tile.TileContext(nc) as tc:
        with tc.tile_pool(name="pw", bufs=1) as pw, \
             tc.tile_pool(name="px", bufs=1) as px, \
             tc.tile_pool(name="pg", bufs=1) as pg, \
             tc.tile_pool(name="pc", bufs=1) as pcp, \
             tc.tile_pool(name="pp", bufs=1, space="PSUM") as pp, \
             tc.tile_pool(name="pd", bufs=1, space="DRAM") as pd:

            # ---------- persistent weights / tables ----------
            brt_sb = []
            bit_sb = []
            ct_sb = []
            cos_sb = []
            sin_sb = []
            rpow_sb = []
            for d in range(NSL):
                t_ = pw.tile([128, H], MM_DT, name=f"brt{d}")
                nc.sync.dma_start(t_[:], brt_d[d * 128:(d + 1) * 128, :])
                brt_sb.append(t_)
                t_ = pw.tile([128, H], MM_DT, name=f"bit{d}")
                nc.sync.dma_start(t_[:], bit_d[d * 128:(d + 1) * 128, :])
                bit_sb.append(t_)
                t_ = pw.tile([128, M], MM_DT, name=f"ct{d}")
                nc.sync.dma_start(t_[:], ct_d[d * 128:(d + 1) * 128, :])
                ct_sb.append(t_)
                t_ = pw.tile([128, T], TABLE_DT, name=f"cos{d}")
                nc.sync.dma_start(t_[:], cos_d[d * 128:(d + 1) * 128, :])
                cos_sb.append(t_)
                t_ = pw.tile([128, T], TABLE_DT, name=f"sin{d}")
                nc.sync.dma_start(t_[:], sin_d[d * 128:(d + 1) * 128, :])
                sin_sb.append(t_)
                t_ = pw.tile([128, T], TABLE_DT, name=f"rpow{d}")
                nc.sync.dma_start(t_[:], rpow_d[d * 128:(d + 1) * 128, :])
                rpow_sb.append(t_)

            ident_sb = pw.tile([128, 128], f32, name="ident")
            nc.sync.dma_start(ident_sb[:], ident_d)
            cwfre_sb = pw.tile([128, 128], f32, name="cwfre")
            nc.sync.dma_start(cwfre_sb[:], cwfre_d)
            cwfim_sb = pw.tile([128, 128], f32, name="cwfim")
            nc.sync.dma_start(cwfim_sb[:], cwfim_d)
            rwfre_sb = pw.tile([128, 128], f32, name="rwfre")
            nc.sync.dma_start(rwfre_sb[:], rwfre_d)
            rwfim_sb = pw.tile([128, 128], f32, name="rwfim")
            nc.sync.dma_start(rwfim_sb[:], rwfim_d)

            # consts: DMA [128,128] then transpose once -> columns
            craw = pw.tile([128, 128], f32, name="craw")
            nc.sync.dma_start(craw[:], consts_d)
            cps = pp.tile([128, 128], f32, name="cps", tag="tp", bufs=2)
            nc.tensor.transpose(cps[:], craw[:], ident_sb[:])
            cT = pw.tile([128, 128], f32, name="cT")
            nc.vector.tensor_copy(cT[:], cps[:])

            def ccv(q, pt):
                # [128, 8] strided view of const block q, part pt (0=re,1=im)
                return cT[:, 16 * q + pt:16 * q + 16:2]

            def ccol(q, sl, pt):
                return cT[:, 16 * q + 2 * sl + pt:16 * q + 2 * sl + pt + 1]

            # persistent carry state (ping-pong via python handles)
            zR_re = pcp.tile([128, 8], f32, name="zR_re")
            zR_im = pcp.tile([128, 8], f32, name="zR_im")
            zW_re = pcp.tile([128, 8], f32, name="zW_re")
            zW_im = pcp.tile([128, 8], f32, name="zW_im")
            nc.vector.memzero(zR_re[:])
            nc.vector.memzero(zR_im[:])
            nc.vector.memzero(zW_re[:])
            nc.vector.memzero(zW_im[:])
            R_re, R_im = zR_re, zR_im
            Wi_re, Wi_im = zW_re, zW_im

            for s in range(S):
                # ---------- load x chunk ----------
                xt_sb = []
                for d in range(NSL):
                    t_ = px.tile([128, T], MM_DT, name=f"xt_s{s}_d{d}",
                                 tag="xt", bufs=16)
                    nc.sync.dma_start(t_[:], xt_d[s, d * 128:(d + 1) * 128, :])
                    xt_sb.append(t_)

                # ---------- Bu matmuls + pre-rotation + scans ----------
                W_re_t = []
                W_im_t = []
                for sl in range(NSL):
                    hs = slice(sl * 128, (sl + 1) * 128)
                    ps_re = pp.tile([128, T], f32, name=f"psre{s}_{sl}",
                                    tag="bure", bufs=2)
                    ps_im = pp.tile([128, T], f32, name=f"psim{s}_{sl}",
                                    tag="buim", bufs=2)
                    for d in range(NSL):
                        nc.tensor.matmul(ps_re[:], brt_sb[d][:, hs], xt_sb[d][:],
                                         start=(d == 0), stop=(d == NSL - 1))
                    for d in range(NSL):
                        nc.tensor.matmul(ps_im[:], bit_sb[d][:, hs], xt_sb[d][:],
                                         start=(d == 0), stop=(d == NSL - 1))
                    # g_re = COS*P_re + SIN*P_im ; g_im = COS*P_im - SIN*P_re
                    t0 = pg.tile([128, T], f32, name=f"t0_{s}_{sl}", tag="t0", bufs=3)
                    t1 = pg.tile([128, T], f32, name=f"t1_{s}_{sl}", tag="t1", bufs=3)
                    gre = pg.tile([128, T], f32, name=f"gre_{s}_{sl}", tag="gre", bufs=3)
                    gim = pg.tile([128, T], f32, name=f"gim_{s}_{sl}", tag="gim", bufs=3)
                    nc.vector.tensor_tensor(t0[:], cos_sb[sl][:], ps_re[:], AL.mult)
                    nc.vector.tensor_tensor(t1[:], sin_sb[sl][:], ps_im[:], AL.mult)
                    nc.vector.tensor_add(gre[:], t0[:], t1[:])
                    nc.vector.tensor_tensor(t0[:], cos_sb[sl][:], ps_im[:], AL.mult)
                    nc.vector.tensor_tensor(t1[:], sin_sb[sl][:], ps_re[:], AL.mult)
                    nc.vector.tensor_sub(gim[:], t0[:], t1[:])
                    # scans
                    wre = pg.tile([128, T], f32, name=f"wre_{s}_{sl}", tag="wre", bufs=10)
                    wim = pg.tile([128, T], f32, name=f"wim_{s}_{sl}", tag="wim", bufs=10)
                    rdec = ccol(5, sl, 1).broadcast_to([128, T])  # r column
                    nc.vector.tensor_tensor_scan(wre[:], rdec, gre[:],
                                                 Wi_re[:, sl:sl + 1], AL.mult, AL.add)
                    nc.vector.tensor_tensor_scan(wim[:], rdec, gim[:],
                                                 Wi_im[:, sl:sl + 1], AL.mult, AL.add)
                    W_re_t.append(wre)
                    W_im_t.append(wim)

                # ---------- E extraction + publish ----------
                wlre = pcp.tile([128, 8], f32, name=f"wlre{s}", tag="wl", bufs=2)
                wlim = pcp.tile([128, 8], f32, name=f"wlim{s}", tag="wl2", bufs=2)
                for sl in range(NSL):
                    nc.vector.tensor_copy(wlre[:, sl:sl + 1], W_re_t[sl][:, T - 1:T])
                    nc.vector.tensor_copy(wlim[:, sl:sl + 1], W_im_t[sl][:, T - 1:T])
                epack = pcp.tile([128, 16], f32, name=f"epack{s}", tag="ep", bufs=2)

                _sc = [0]

                def t8(a, b, op):
                    # fresh [128,8] scratch per op to avoid aliasing
                    _sc[0] += 1
                    out = pcp.tile([128, 8], f32, name=f"sc{s}_{_sc[0]}",
                                   tag=f"sc{_sc[0] % 12}", bufs=2)
                    nc.vector.tensor_tensor(out[:], a, b, op)
                    return out[:]

                def cmul(wre_v, wim_v, zre, zim):
                    # complex (wre_v + i wim_v) * (zre + i zim); args are APs
                    re = t8(t8(wre_v, zre, AL.mult), t8(wim_v, zim, AL.mult),
                            AL.subtract)
                    im = t8(t8(wre_v, zim, AL.mult), t8(wim_v, zre, AL.mult),
                            AL.add)
                    return re, im

                # E = ROTT1*Wl - QC1*R_prev
                e1re, e1im = cmul(ccv(2, 0), ccv(2, 1), wlre[:], wlim[:])
                e2re, e2im = cmul(ccv(1, 0), ccv(1, 1), R_re[:], R_im[:])
                nc.vector.tensor_tensor(epack[:, 0:16:2], e1re, e2re, AL.subtract)
                nc.vector.tensor_tensor(epack[:, 1:16:2], e1im, e2im, AL.subtract)

                # transpose epack [128,16] -> [16,128], publish, AllGather
                pub_ps = pp.tile([16, 128], f32, name=f"pubps{s}", tag="tp", bufs=2)
                nc.tensor.transpose(pub_ps[:], epack[:], ident_sb[:])
                pub_sb = pcp.tile([16, 128], f32, name=f"pubsb{s}", tag="pub", bufs=2)
                nc.vector.tensor_copy(pub_sb[:], pub_ps[:])
                pub_dr = pd.tile([16, 128], f32, name=f"pubdr{s}", tag="pubd", bufs=2)
                nc.sync.dma_start(pub_dr[:], pub_sb[:])
                gat_dr = pd.tile([128, 128], f32, name=f"gatdr{s}", tag="gatd",
                                 bufs=2, addr_space="Shared")
                nc.gpsimd.collective_compute(
                    "AllGather", AL.bypass,
                    replica_groups=[list(range(NC))],
                    ins=[pub_dr[:].opt()],
                    outs=[gat_dr[:].opt()],
                )
                eg = pcp.tile([128, 128], f32, name=f"eg{s}", tag="eg", bufs=2)
                nc.sync.dma_start(eg[:], gat_dr[:])
                et_ps = pp.tile([128, 128], f32, name=f"etps{s}", tag="tp", bufs=2)
                nc.tensor.transpose(et_ps[:], eg[:], ident_sb[:])
                et = pcp.tile([128, 128], f32, name=f"et{s}", tag="et", bufs=2)
                nc.vector.tensor_copy(et[:], et_ps[:])

                # ---------- V (same-slot carry) and R update ----------
                def wsum(fold_sb, nmv, nmr1, nm):
                    tmp = pcp.tile([128, 128], f32, name=f"{nmv}{s}", tag="redt", bufs=2)
                    nc.vector.tensor_tensor(tmp[:], fold_sb[:], et[:], AL.mult)
                    red1 = pcp.tile([128, 16], f32, name=f"{nmr1}{s}", tag="red1", bufs=2)
                    nc.vector.tensor_reduce(
                        red1[:].unsqueeze(2),
                        tmp[:].rearrange("p (j x) -> p x j", j=8),
                        AX.X, AL.add)
                    out = pcp.tile([128, 8], f32, name=f"{nm}{s}", tag=nm, bufs=2)
                    nc.vector.tensor_reduce(
                        out[:].unsqueeze(2),
                        red1[:].rearrange("p (sl pt) -> p sl pt", pt=2),
                        AX.X, AL.add)
                    return out

                v_re = wsum(cwfre_sb, "tmpa", "reda", "vre")
                v_im = wsum(cwfim_sb, "tmpb", "redb", "vim")
                rp_re = wsum(rwfre_sb, "tmpc", "redc", "rpre")
                rp_im = wsum(rwfim_sb, "tmpd", "redd", "rpim")

                # V' = ROT1 * V
                vp_re, vp_im = cmul(ccv(3, 0), ccv(3, 1), v_re[:], v_im[:])

                # R_new = Q8*R + Rpart ; Winit_next = WC * R_new
                q8r_re, q8r_im = cmul(ccv(4, 0), ccv(4, 1), R_re[:], R_im[:])
                rn_re = pcp.tile([128, 8], f32, name=f"rnre{s}", tag="rn", bufs=2)
                rn_im = pcp.tile([128, 8], f32, name=f"rnim{s}", tag="rn2", bufs=2)
                nc.vector.tensor_add(rn_re[:], q8r_re, rp_re[:])
                nc.vector.tensor_add(rn_im[:], q8r_im, rp_im[:])
                R_re, R_im = rn_re, rn_im
                wi2_re = pcp.tile([128, 8], f32, name=f"wire{s}", tag="wi", bufs=2)
                wi2_im = pcp.tile([128, 8], f32, name=f"wiim{s}", tag="wi2", bufs=2)
                wt_re, wt_im = cmul(ccv(0, 0), ccv(0, 1), rn_re[:], rn_im[:])
                nc.vector.tensor_copy(wi2_re[:], wt_re)
                nc.vector.tensor_copy(wi2_im[:], wt_im)
                Wi_re, Wi_im = wi2_re, wi2_im

                # ---------- correction + post-rotation ----------
                u_t = []
                for sl in range(NSL):
                    wtre = pg.tile([128, T], f32, name=f"wtre{s}_{sl}", tag="wtre", bufs=3)
                    wtim = pg.tile([128, T], f32, name=f"wtim{s}_{sl}", tag="wtim", bufs=3)
                    nc.vector.scalar_tensor_tensor(
                        wtre[:], rpow_sb[sl][:], vp_re[:, sl:sl + 1], W_re_t[sl][:],
                        AL.mult, AL.add)
                    nc.vector.scalar_tensor_tensor(
                        wtim[:], rpow_sb[sl][:], vp_im[:, sl:sl + 1], W_im_t[sl][:],
                        AL.mult, AL.add)
                    p0 = pg.tile([128, T], f32, name=f"p0_{s}_{sl}", tag="p0", bufs=3)
                    p1 = pg.tile([128, T], f32, name=f"p1_{s}_{sl}", tag="p1", bufs=3)
                    nc.vector.tensor_tensor(p0[:], cos_sb[sl][:], wtre[:], AL.mult)
                    nc.vector.tensor_tensor(p1[:], sin_sb[sl][:], wtim[:], AL.mult)
                    u = pg.tile([128, T], MM_DT, name=f"u{s}_{sl}", tag="u", bufs=10)
                    nc.vector.tensor_sub(u[:], p0[:], p1[:])
                    u_t.append(u)

                # ---------- output projection ----------
                for n in range(NSL):
                    ns = slice(n * 128, (n + 1) * 128)
                    psy = pp.tile([128, T], f32, name=f"psy{s}_{n}", tag="ytile", bufs=2)
                    for sl in range(NSL):
                        nc.tensor.matmul(psy[:], ct_sb[sl][:, ns], u_t[sl][:],
                                         start=(sl == 0), stop=(sl == NSL - 1))
                    dx = pg.tile([128, T], f32, name=f"dx{s}_{n}", tag="dx", bufs=2)
                    nc.scalar.mul(dx[:], xt_sb[n][:], ccol(5, n, 0))
                    yo = pg.tile([128, T], f32, name=f"yo{s}_{n}", tag="yo", bufs=2)
                    nc.vector.tensor_tensor(yo[:], dx[:], psy[:], AL.add)
                    nc.sync.dma_start(y_d[s, ns, :], yo[:])

    nc.compile()
    _BUILD_CACHE["nc"] = nc
    return nc


def _prep(inputs, A_re, A_im, B_re, B_im, C, D):
    x = np.asarray(inputs)
    A = A_re.astype(np.float64) + 1j * A_im.astype(np.float64)
    r = np.abs(A)
    th = np.angle(A)
    k = np.arange(T)
    COS = np.cos(th[:, None] * k)
    SIN = np.sin(th[:, None] * k)
    RPOW = r[:, None] ** (k + 1)
    Q = A ** T
    ROT1 = np.exp(1j * th)
    ROTT1 = np.exp(1j * th * (T - 1))
    Q8 = Q ** 8
    RW = [Q ** (7 - j) for j in range(NC)]

    np16 = np.float16 if MM_DT == f16 else np.float32
    tb16 = np.float16 if TABLE_DT == f16 else np.float32

    brt = np.ascontiguousarray(B_re.T).astype(np16)
    bit = np.ascontiguousarray(B_im.T).astype(np16)
    ct = np.ascontiguousarray(C.T).astype(np16)
    cos_t = COS.astype(tb16)
    sin_t = SIN.astype(tb16)
    rpow_t = RPOW.astype(tb16)
    ident = np.eye(128, dtype=np.float32)

    xT = np.ascontiguousarray(x.T)  # [M, L]

    def cvec_rows(z):
        # complex [H] -> rows [16, 128] (row = 2*sl + pt)
        out = np.zeros((16, 128), np.float32)
        zr = z.real.astype(np.float32).reshape(8, 128)
        zi = z.imag.astype(np.float32).reshape(8, 128)
        out[0::2] = zr
        out[1::2] = zi
        return out

    rwf_re = np.zeros((128, 128), np.float32)
    rwf_im = np.zeros((128, 128), np.float32)
    for j in range(NC):
        w = RW[j]
        wr = w.real.astype(np.float32).reshape(8, 128)
        wi = w.imag.astype(np.float32).reshape(8, 128)
        for sl in range(8):
            rwf_re[:, 16 * j + 2 * sl + 0] = wr[sl]
            rwf_re[:, 16 * j + 2 * sl + 1] = -wi[sl]
            rwf_im[:, 16 * j + 2 * sl + 0] = wi[sl]
            rwf_im[:, 16 * j + 2 * sl + 1] = wr[sl]

    in_maps = []
    for c in range(NC):
        WC = ROT1 * Q ** c
        QC1 = Q ** (c + 1)
        consts = np.zeros((128, 128), np.float32)
        consts[0:16] = cvec_rows(WC)
        consts[16:32] = cvec_rows(QC1)
        consts[32:48] = cvec_rows(ROTT1)
        consts[48:64] = cvec_rows(ROT1)
        consts[64:80] = cvec_rows(Q8)
        # block 5: rows 80..95: row 80+2*sl = D slice, row 81+2*sl = r slice
        consts[80:96] = cvec_rows(D.astype(np.float64) + 1j * r)

        cwf_re = np.zeros((128, 128), np.float32)
        cwf_im = np.zeros((128, 128), np.float32)
        for j in range(c):
            w = Q ** (c - 1 - j)
            wr = w.real.astype(np.float32).reshape(8, 128)
            wi = w.imag.astype(np.float32).reshape(8, 128)
            for sl in range(8):
                cwf_re[:, 16 * j + 2 * sl + 0] = wr[sl]
                cwf_re[:, 16 * j + 2 * sl + 1] = -wi[sl]
                cwf_im[:, 16 * j + 2 * sl + 0] = wi[sl]
                cwf_im[:, 16 * j + 2 * sl + 1] = wr[sl]

        xt = np.zeros((S, M, T), np16)
        for s in range(S):
            m = 8 * s + c
            xt[s] = xT[:, m * T:(m + 1) * T]

        in_maps.append({
            "xt": xt, "brt": brt, "bit": bit, "ct": ct,
            "costb": cos_t, "sintb": sin_t, "rpowtb": rpow_t,
            "consts": consts,
            "cwfre": cwf_re, "cwfim": cwf_im,
            "rwfre": rwf_re, "rwfim": rwf_im,
            "ident": ident,
        })
    return in_maps


LAST_RESULTS = {}


def kernel(inputs, A_re, A_im, B_re, B_im, C, D):
    nc = _build()
    in_maps = _prep(inputs, A_re, A_im, B_re, B_im, C, D)
    trace = bool(os.environ.get("DIAG_TRACE"))
    res = run_bass_kernel_spmd(nc, in_maps, core_ids=list(range(NC)),
                               trace=trace)
    LAST_RESULTS["exec_time_ns"] = res.exec_time_ns
    LAST_RESULTS["mean_exec_time_ns"] = res.mean_exec_time_ns
    yT = np.zeros((M, L), np.float32)
    for c in range(NC):
        yc = res.results[c]["y"]
        for s in range(S):
            m = 8 * s + c
            yT[:, m * T:(m + 1) * T] = yc[s]
    return np.ascontiguousarray(yT.T)


# revision 5
# speedup vs baseline: 1.1470x; 1.1470x over previous
"""Trainium2 Bass kernel for nn_DiagRNN (diagonal complex linear RNN / LRU).

  y = Re[C @ h] + D*x,  h_t = A h_{t-1} + B x_t  (A complex-diagonal)

Strategy (8 NeuronCores, sequence-parallel):
  * Sequence of L=16384 split into 32 chunks of T=512. Chunk m is processed
    by core m%8 in "slot" m//8 (interleaved assignment) so cross-core carry
    exchange is a small per-slot AllGather that pipelines behind compute.
  * Complex scan is reduced to two REAL first-order scans per chunk via a
    rotating-frame transform: with A = r*e^{i\theta},
        W_k = e^{-i\theta k} h_{mT+k}  satisfies  W_k = r W_{k-1} + g_k,
        g_k = e^{-i\theta k} (B x)_{mT+k}.
    The real scans run on the DVE hardware scan instruction
    (tensor_tensor_scan).  Pre/post rotations are elementwise with
    host-precomputed cos/sin/r^k tables.
  * Carries: cores publish standalone chunk sums E_m, AllGather them, and
    each core folds same-slot predecessors in with one fused
    scalar_tensor_tensor per (slice, re/im); cross-slot history enters for
    free through the scan initial value.
  * Matmuls (B_re, B_im, C projections) run on the PE at 1 cycle/row using
    fp16 operands (B/x) and fp16 C/u; accumulation is fp32 in PSUM.
"""
import sys, os
sys.path.insert(0, '/opt/trn_rl_repo')
import numpy as np

import concourse.bass as bass
import concourse.bacc as bacc
import concourse.tile as tile
import concourse.mybir as mybir
from concourse.bass_utils import run_bass_kernel_spmd

L, H, M = 16384, 1024, 1024
NC = 8
T = 512
S = L // (T * NC)          # 4 slots
NSL = H // 128             # 8 slices

f32 = mybir.dt.float32
f32r = mybir.dt.float32r
f16 = mybir.dt.float16
AL = mybir.AluOpType
AX = mybir.AxisListType

TABLE_DT = f16   # cos/sin/rpow tables
MM_DT = f16      # B, x, C, u matmul operand dtype

_BUILD_CACHE = {}


def _build():
    if "nc" in _BUILD_CACHE:
        return _BUILD_CACHE["nc"]
    nc = bacc.Bacc("TRN2", target_bir_lowering=False, debug=False,
                   num_devices=NC)

    xt_d = nc.dram_tensor("xt", [S, M, T], MM_DT, kind="ExternalInput").ap()
    brt_d = nc.dram_tensor("brt", [M, H], MM_DT, kind="ExternalInput").ap()
    bit_d = nc.dram_tensor("bit", [M, H], MM_DT, kind="ExternalInput").ap()
    ct_d = nc.dram_tensor("ct", [H, M], MM_DT, kind="ExternalInput").ap()
    cos_d = nc.dram_tensor("costb", [H, T], TABLE_DT, kind="ExternalInput").ap()
    sin_d = nc.dram_tensor("sintb", [H, T], TABLE_DT, kind="ExternalInput").ap()
    rpow_d = nc.dram_tensor("rpowtb", [H, T], TABLE_DT, kind="ExternalInput").ap()
    consts_d = nc.dram_tensor("consts", [128, 128], f32, kind="ExternalInput").ap()
    cwfre_d = nc.dram_tensor("cwfre", [128, 128], f32, kind="ExternalInput").ap()
    cwfim_d = nc.dram_tensor("cwfim", [128, 128], f32, kind="ExternalInput").ap()
    rwfre_d = nc.dram_tensor("rwfre", [128, 128], f32, kind="ExternalInput").ap()
    rwfim_d = nc.dram_tensor("rwfim", [128, 128], f32, kind="ExternalInput").ap()
    ident_d = nc.dram_tensor("ident", [128, 128], f32, kind="ExternalInput").ap()
    y_d = nc.dram_tensor("y", [S, M, T], f32, kind="ExternalOutput").ap()

    with tile.TileContext(nc) as tc:
        with tc.tile_pool(name="pw", bufs=1) as pw, \
             tc.tile_pool(name="px", bufs=1) as px, \
             tc.tile_pool(name="pg", bufs=1) as pg, \
             tc.tile_pool(name="pc", bufs=1) as pcp, \
             tc.tile_pool(name="pp", bufs=1, space="PSUM") as pp, \
             tc.tile_pool(name="pd", bufs=1, space="DRAM") as pd:

            # ---------- persistent weights / tables ----------
            brt_sb = []
            bit_sb = []
            ct_sb = []
            cos_sb = []
            sin_sb = []
            rpow_sb = []
            for d in range(NSL):
                t_ = pw.tile([128, H], MM_DT, name=f"brt{d}")
                nc.sync.dma_start(t_[:], brt_d[d * 128:(d + 1) * 128, :])
                brt_sb.append(t_)
                t_ = pw.tile([128, H], MM_DT, name=f"bit{d}")
                nc.sync.dma_start(t_[:], bit_d[d * 128:(d + 1) * 128, :])
                bit_sb.append(t_)
                t_ = pw.tile([128, M], MM_DT, name=f"ct{d}")
                nc.sync.dma_start(t_[:], ct_d[d * 128:(d + 1) * 128, :])
                ct_sb.append(t_)
                t_ = pw.tile([128, T], TABLE_DT, name=f"cos{d}")
                nc.sync.dma_start(t_[:], cos_d[d * 128:(d + 1) * 128, :])
                cos_sb.append(t_)
                t_ = pw.tile([128, T], TABLE_DT, name=f"sin{d}")
                nc.sync.dma_start(t_[:], sin_d[d * 128:(d + 1) * 128, :])
                sin_sb.append(t_)
                t_ = pw.tile([128, T], TABLE_DT, name=f"rpow{d}")
                nc.sync.dma_start(t_[:], rpow_d[d * 128:(d + 1) * 128, :])
                rpow_sb.append(t_)

            ident_sb = pw.tile([128, 128], f32, name="ident")
            nc.sync.dma_start(ident_sb[:], ident_d)
            cwfre_sb = pw.tile([128, 128], f32, name="cwfre")
            nc.sync.dma_start(cwfre_sb[:], cwfre_d)
            cwfim_sb = pw.tile([128, 128], f32, name="cwfim")
            nc.sync.dma_start(cwfim_sb[:], cwfim_d)
            rwfre_sb = pw.tile([128, 128], f32, name="rwfre")
            nc.sync.dma_start(rwfre_sb[:], rwfre_d)
            rwfim_sb = pw.tile([128, 128], f32, name="rwfim")
            nc.sync.dma_start(rwfim_sb[:], rwfim_d)

            # consts: DMA [128,128] then transpose once -> columns
            craw = pw.tile([128, 128], f32, name="craw")
            nc.sync.dma_start(craw[:], consts_d)
            cps = pp.tile([128, 128], f32, name="cps", tag="tp", bufs=2)
            nc.tensor.transpose(cps[:], craw[:], ident_sb[:])
            cT = pw.tile([128, 128], f32, name="cT")
            nc.vector.tensor_copy(cT[:], cps[:])

            def ccv(q, pt):
                # [128, 8] strided view of const block q, part pt (0=re,1=im)
                return cT[:, 16 * q + pt:16 * q + 16:2]

            def ccol(q, sl, pt):
                return cT[:, 16 * q + 2 * sl + pt:16 * q + 2 * sl + pt + 1]

            # persistent carry state (ping-pong via python handles)
            zR_re = pcp.tile([128, 8], f32, name="zR_re")
            zR_im = pcp.tile([128, 8], f32, name="zR_im")
            zW_re = pcp.tile([128, 8], f32, name="zW_re")
            zW_im = pcp.tile([128, 8], f32, name="zW_im")
            nc.vector.memzero(zR_re[:])
            nc.vector.memzero(zR_im[:])
            nc.vector.memzero(zW_re[:])
            nc.vector.memzero(zW_im[:])
            R_re, R_im = zR_re, zR_im
            Wi_re, Wi_im = zW_re, zW_im

            for s in range(S):
                # ---------- load x chunk ----------
                xt_sb = []
                for d in range(NSL):
                    t_ = px.tile([128, T], MM_DT, name=f"xt_s{s}_d{d}",
                                 tag="xt", bufs=16)
                    nc.sync.dma_start(t_[:], xt_d[s, d * 128:(d + 1) * 128, :])
                    xt_sb.append(t_)

                # ---------- Bu matmuls + pre-rotation + scans ----------
                W_re_t = []
                W_im_t = []
                for sl in range(NSL):
                    hs = slice(sl * 128, (sl + 1) * 128)
                    ps_re = pp.tile([128, T], f32, name=f"psre{s}_{sl}",
                                    tag="bure", bufs=2)
                    ps_im = pp.tile([128, T], f32, name=f"psim{s}_{sl}",
                                    tag="buim", bufs=2)
                    for d in range(NSL):
                        nc.tensor.matmul(ps_re[:], brt_sb[d][:, hs], xt_sb[d][:],
                                         start=(d == 0), stop=(d == NSL - 1))
                    for d in range(NSL):
                        nc.tensor.matmul(ps_im[:], bit_sb[d][:, hs], xt_sb[d][:],
                                         start=(d == 0), stop=(d == NSL - 1))
                    # evict PSUM via idle ACT engine to fp16
                    pre16 = pg.tile([128, T], f16, name=f"pre16_{s}_{sl}", tag="pre16", bufs=3)
                    pim16 = pg.tile([128, T], f16, name=f"pim16_{s}_{sl}", tag="pim16", bufs=3)
                    nc.scalar.copy(pre16[:], ps_re[:])
                    nc.scalar.copy(pim16[:], ps_im[:])
                    # g_re = COS*P_re + SIN*P_im ; g_im = COS*P_im - SIN*P_re
                    t0 = pg.tile([128, T], f16, name=f"t0_{s}_{sl}", tag="t0", bufs=3)
                    t1 = pg.tile([128, T], f16, name=f"t1_{s}_{sl}", tag="t1", bufs=3)
                    t2 = pg.tile([128, T], f16, name=f"t2_{s}_{sl}", tag="t2", bufs=3)
                    t3 = pg.tile([128, T], f16, name=f"t3_{s}_{sl}", tag="t3", bufs=3)
                    gre = pg.tile([128, T], f16, name=f"gre_{s}_{sl}", tag="gre", bufs=3)
                    gim = pg.tile([128, T], f16, name=f"gim_{s}_{sl}", tag="gim", bufs=3)
                    nc.vector.tensor_tensor(t0[:], cos_sb[sl][:], pre16[:], AL.mult)
                    nc.vector.tensor_tensor(t1[:], sin_sb[sl][:], pim16[:], AL.mult)
                    nc.vector.tensor_add(gre[:], t0[:], t1[:])
                    nc.vector.tensor_tensor(t2[:], cos_sb[sl][:], pim16[:], AL.mult)
                    nc.vector.tensor_tensor(t3[:], sin_sb[sl][:], pre16[:], AL.mult)
                    nc.vector.tensor_sub(gim[:], t2[:], t3[:])
                    # scans
                    wre = pg.tile([128, T], f16, name=f"wre_{s}_{sl}", tag="wre", bufs=12)
                    wim = pg.tile([128, T], f16, name=f"wim_{s}_{sl}", tag="wim", bufs=12)
                    rdec = ccol(5, sl, 1).broadcast_to([128, T])  # r column
                    nc.vector.tensor_tensor_scan(wre[:], rdec, gre[:],
                                                 Wi_re[:, sl:sl + 1], AL.mult, AL.add)
                    nc.vector.tensor_tensor_scan(wim[:], rdec, gim[:],
                                                 Wi_im[:, sl:sl + 1], AL.mult, AL.add)
                    W_re_t.append(wre)
                    W_im_t.append(wim)

                # ---------- E extraction + publish ----------
                wlre = pcp.tile([128, 8], f32, name=f"wlre{s}", tag="wl", bufs=2)
                wlim = pcp.tile([128, 8], f32, name=f"wlim{s}", tag="wl2", bufs=2)
                for sl in range(NSL):
                    nc.scalar.copy(wlre[:, sl:sl + 1], W_re_t[sl][:, T - 1:T])
                    nc.scalar.copy(wlim[:, sl:sl + 1], W_im_t[sl][:, T - 1:T])
                epack = pcp.tile([128, 16], f32, name=f"epack{s}", tag="ep", bufs=2)

                _sc = [0]

                def t8(a, b, op):
                    # fresh [128,8] scratch per op to avoid aliasing
                    _sc[0] += 1
                    out = pcp.tile([128, 8], f32, name=f"sc{s}_{_sc[0]}",
                                   tag=f"sc{_sc[0] % 12}", bufs=2)
                    nc.gpsimd.tensor_tensor(out[:], a, b, op)
                    return out[:]

                def cmul(wre_v, wim_v, zre, zim):
                    # complex (wre_v + i wim_v) * (zre + i zim); args are APs
                    re = t8(t8(wre_v, zre, AL.mult), t8(wim_v, zim, AL.mult),
                            AL.subtract)
                    im = t8(t8(wre_v, zim, AL.mult), t8(wim_v, zre, AL.mult),
                            AL.add)
                    return re, im

                # E = ROTT1*Wl - QC1*R_prev
                e1re, e1im = cmul(ccv(2, 0), ccv(2, 1), wlre[:], wlim[:])
                e2re, e2im = cmul(ccv(1, 0), ccv(1, 1), R_re[:], R_im[:])
                nc.gpsimd.tensor_tensor(epack[:, 0:16:2], e1re, e2re, AL.subtract)
                nc.gpsimd.tensor_tensor(epack[:, 1:16:2], e1im, e2im, AL.subtract)

                # transpose epack [128,16] -> [16,128], publish, AllGather
                pub_ps = pp.tile([16, 128], f32, name=f"pubps{s}", tag="tp", bufs=2)
                nc.tensor.transpose(pub_ps[:], epack[:], ident_sb[:])
                pub_sb = pcp.tile([16, 128], f32, name=f"pubsb{s}", tag="pub", bufs=2)
                nc.vector.tensor_copy(pub_sb[:], pub_ps[:])
                pub_dr = pd.tile([16, 128], f32, name=f"pubdr{s}", tag="pubd", bufs=2)
                nc.sync.dma_start(pub_dr[:], pub_sb[:])
                gat_dr = pd.tile([128, 128], f32, name=f"gatdr{s}", tag="gatd",
                                 bufs=2, addr_space="Shared")
                nc.gpsimd.collective_compute(
                    "AllGather", AL.bypass,
                    replica_groups=[list(range(NC))],
                    ins=[pub_dr[:].opt()],
                    outs=[gat_dr[:].opt()],
                )
                eg = pcp.tile([128, 128], f32, name=f"eg{s}", tag="eg", bufs=2)
                nc.sync.dma_start(eg[:], gat_dr[:])
                et_ps = pp.tile([128, 128], f32, name=f"etps{s}", tag="tp", bufs=2)
                nc.tensor.transpose(et_ps[:], eg[:], ident_sb[:])
                et = pcp.tile([128, 128], f32, name=f"et{s}", tag="et", bufs=2)
                nc.vector.tensor_copy(et[:], et_ps[:])

                # ---------- V (same-slot carry) and R update ----------
                def wsum(fold_sb, nmv, nmr1, nm):
                    tmp = pcp.tile([128, 128], f32, name=f"{nmv}{s}", tag="redt", bufs=2)
                    nc.vector.tensor_tensor(tmp[:], fold_sb[:], et[:], AL.mult)
                    red1 = pcp.tile([128, 16], f32, name=f"{nmr1}{s}", tag="red1", bufs=2)
                    nc.vector.tensor_reduce(
                        red1[:].unsqueeze(2),
                        tmp[:].rearrange("p (j x) -> p x j", j=8),
                        AX.X, AL.add)
                    out = pcp.tile([128, 8], f32, name=f"{nm}{s}", tag=nm, bufs=2)
                    nc.vector.tensor_reduce(
                        out[:].unsqueeze(2),
                        red1[:].rearrange("p (sl pt) -> p sl pt", pt=2),
                        AX.X, AL.add)
                    return out

                v_re = wsum(cwfre_sb, "tmpa", "reda", "vre")
                v_im = wsum(cwfim_sb, "tmpb", "redb", "vim")
                rp_re = wsum(rwfre_sb, "tmpc", "redc", "rpre")
                rp_im = wsum(rwfim_sb, "tmpd", "redd", "rpim")

                # V' = ROT1 * V
                vp_re, vp_im = cmul(ccv(3, 0), ccv(3, 1), v_re[:], v_im[:])

                # R_new = Q8*R + Rpart ; Winit_next = WC * R_new
                q8r_re, q8r_im = cmul(ccv(4, 0), ccv(4, 1), R_re[:], R_im[:])
                rn_re = pcp.tile([128, 8], f32, name=f"rnre{s}", tag="rn", bufs=2)
                rn_im = pcp.tile([128, 8], f32, name=f"rnim{s}", tag="rn2", bufs=2)
                nc.vector.tensor_add(rn_re[:], q8r_re, rp_re[:])
                nc.vector.tensor_add(rn_im[:], q8r_im, rp_im[:])
                R_re, R_im = rn_re, rn_im
                wi2_re = pcp.tile([128, 8], f32, name=f"wire{s}", tag="wi", bufs=2)
                wi2_im = pcp.tile([128, 8], f32, name=f"wiim{s}", tag="wi2", bufs=2)
                wt_re, wt_im = cmul(ccv(0, 0), ccv(0, 1), rn_re[:], rn_im[:])
                nc.vector.tensor_copy(wi2_re[:], wt_re)
                nc.vector.tensor_copy(wi2_im[:], wt_im)
                Wi_re, Wi_im = wi2_re, wi2_im

                # ---------- correction + post-rotation ----------
                u_t = []
                for sl in range(NSL):
                    wtre = pg.tile([128, T], f16, name=f"wtre{s}_{sl}", tag="wtre", bufs=3)
                    wtim = pg.tile([128, T], f16, name=f"wtim{s}_{sl}", tag="wtim", bufs=3)
                    nc.vector.scalar_tensor_tensor(
                        wtre[:], rpow_sb[sl][:], vp_re[:, sl:sl + 1], W_re_t[sl][:],
                        AL.mult, AL.add)
                    nc.vector.scalar_tensor_tensor(
                        wtim[:], rpow_sb[sl][:], vp_im[:, sl:sl + 1], W_im_t[sl][:],
                        AL.mult, AL.add)
                    p0 = pg.tile([128, T], f16, name=f"p0_{s}_{sl}", tag="p0", bufs=3)
                    p1 = pg.tile([128, T], f16, name=f"p1_{s}_{sl}", tag="p1", bufs=3)
                    nc.vector.tensor_tensor(p0[:], cos_sb[sl][:], wtre[:], AL.mult)
                    nc.vector.tensor_tensor(p1[:], sin_sb[sl][:], wtim[:], AL.mult)
                    u = pg.tile([128, T], MM_DT, name=f"u{s}_{sl}", tag="u", bufs=10)
                    nc.vector.tensor_sub(u[:], p0[:], p1[:])
                    u_t.append(u)

                # ---------- output projection ----------
                for n in range(NSL):
                    ns = slice(n * 128, (n + 1) * 128)
                    psy = pp.tile([128, T], f32, name=f"psy{s}_{n}", tag="ytile", bufs=2)
                    for sl in range(NSL):
                        nc.tensor.matmul(psy[:], ct_sb[sl][:, ns], u_t[sl][:],
                                         start=(sl == 0), stop=(sl == NSL - 1))
                    yo = pg.tile([128, T], f32, name=f"yo{s}_{n}", tag="yo", bufs=2)
                    nc.vector.scalar_tensor_tensor(yo[:], xt_sb[n][:], ccol(5, n, 0),
                                                   psy[:], AL.mult, AL.add)
                    nc.sync.dma_start(y_d[s, ns, :], yo[:])

    nc.compile()
    _BUILD_CACHE["nc"] = nc
    return nc


def _prep(inputs, A_re, A_im, B_re, B_im, C, D):
    x = np.asarray(inputs)
    A = A_re.astype(np.float64) + 1j * A_im.astype(np.float64)
    r = np.abs(A)
    th = np.angle(A)
    k = np.arange(T)
    COS = np.cos(th[:, None] * k)
    SIN = np.sin(th[:, None] * k)
    RPOW = r[:, None] ** (k + 1)
    Q = A ** T
    ROT1 = np.exp(1j * th)
    ROTT1 = np.exp(1j * th * (T - 1))
    Q8 = Q ** 8
    RW = [Q ** (7 - j) for j in range(NC)]

    np16 = np.float16 if MM_DT == f16 else np.float32
    tb16 = np.float16 if TABLE_DT == f16 else np.float32

    brt = np.ascontiguousarray(B_re.T).astype(np16)
    bit = np.ascontiguousarray(B_im.T).astype(np16)
    ct = np.ascontiguousarray(C.T).astype(np16)
    cos_t = COS.astype(tb16)
    sin_t = SIN.astype(tb16)
    rpow_t = RPOW.astype(tb16)
    ident = np.eye(128, dtype=np.float32)

    xT = np.ascontiguousarray(x.T)  # [M, L]

    def cvec_rows(z):
        # complex [H] -> rows [16, 128] (row = 2*sl + pt)
        out = np.zeros((16, 128), np.float32)
        zr = z.real.astype(np.float32).reshape(8, 128)
        zi = z.imag.astype(np.float32).reshape(8, 128)
        out[0::2] = zr
        out[1::2] = zi
        return out

    rwf_re = np.zeros((128, 128), np.float32)
    rwf_im = np.zeros((128, 128), np.float32)
    for j in range(NC):
        w = RW[j]
        wr = w.real.astype(np.float32).reshape(8, 128)
        wi = w.imag.astype(np.float32).reshape(8, 128)
        for sl in range(8):
            rwf_re[:, 16 * j + 2 * sl + 0] = wr[sl]
            rwf_re[:, 16 * j + 2 * sl + 1] = -wi[sl]
            rwf_im[:, 16 * j + 2 * sl + 0] = wi[sl]
            rwf_im[:, 16 * j + 2 * sl + 1] = wr[sl]

    in_maps = []
    for c in range(NC):
        WC = ROT1 * Q ** c
        QC1 = Q ** (c + 1)
        consts = np.zeros((128, 128), np.float32)
        consts[0:16] = cvec_rows(WC)
        consts[16:32] = cvec_rows(QC1)
        consts[32:48] = cvec_rows(ROTT1)
        consts[48:64] = cvec_rows(ROT1)
        consts[64:80] = cvec_rows(Q8)
        # block 5: rows 80..95: row 80+2*sl = D slice, row 81+2*sl = r slice
        consts[80:96] = cvec_rows(D.astype(np.float64) + 1j * r)

        cwf_re = np.zeros((128, 128), np.float32)
        cwf_im = np.zeros((128, 128), np.float32)
        for j in range(c):
            w = Q ** (c - 1 - j)
            wr = w.real.astype(np.float32).reshape(8, 128)
            wi = w.imag.astype(np.float32).reshape(8, 128)
            for sl in range(8):
                cwf_re[:, 16 * j + 2 * sl + 0] = wr[sl]
                cwf_re[:, 16 * j + 2 * sl + 1] = -wi[sl]
                cwf_im[:, 16 * j + 2 * sl + 0] = wi[sl]
                cwf_im[:, 16 * j + 2 * sl + 1] = wr[sl]

        xt = np.zeros((S, M, T), np16)
        for s in range(S):
            m = 8 * s + c
            xt[s] = xT[:, m * T:(m + 1) * T]

        in_maps.append({
            "xt": xt, "brt": brt, "bit": bit, "ct": ct,
            "costb": cos_t, "sintb": sin_t, "rpowtb": rpow_t,
            "consts": consts,
            "cwfre": cwf_re, "cwfim": cwf_im,
            "rwfre": rwf_re, "rwfim": rwf_im,
            "ident": ident,
        })
    return in_maps


LAST_RESULTS = {}


def kernel(inputs, A_re, A_im, B_re, B_im, C, D):
    nc = _build()
    in_maps = _prep(inputs, A_re, A_im, B_re, B_im, C, D)
    trace = bool(os.environ.get("DIAG_TRACE"))
    res = run_bass_kernel_spmd(nc, in_maps, core_ids=list(range(NC)),
                               trace=trace)
    LAST_RESULTS["exec_time_ns"] = res.exec_time_ns
    LAST_RESULTS["mean_exec_time_ns"] = res.mean_exec_time_ns
    yT = np.zeros((M, L), np.float32)
    for c in range(NC):
        yc = res.results[c]["y"]
        for s in range(S):
            m = 8 * s + c
            yT[:, m * T:(m + 1) * T] = yc[s]
    return np.ascontiguousarray(yT.T)


# revision 7
# speedup vs baseline: 1.5052x; 1.3123x over previous
"""Trainium2 Bass kernel for nn_DiagRNN (diagonal complex linear RNN / LRU).

  y = Re[C @ h] + D*x,  h_t = A h_{t-1} + B x_t  (A complex-diagonal)

Strategy (8 NeuronCores, sequence-parallel):
  * Sequence of L=16384 split into 32 chunks of T=512. Chunk m is processed
    by core m%8 in "slot" m//8 (interleaved assignment) so cross-core carry
    exchange is a small per-slot AllGather that pipelines behind compute.
  * Complex scan is reduced to two REAL first-order scans per chunk via a
    rotating-frame transform: with A = r*e^{i\theta},
        W_k = e^{-i\theta k} h_{mT+k}  satisfies  W_k = r W_{k-1} + g_k,
        g_k = e^{-i\theta k} (B x)_{mT+k}.
    The real scans run on the DVE hardware scan instruction
    (tensor_tensor_scan).  Pre/post rotations are elementwise with
    host-precomputed cos/sin/r^k tables.
  * Carries: cores publish standalone chunk sums E_m, AllGather them, and
    each core folds same-slot predecessors in with one fused
    scalar_tensor_tensor per (slice, re/im); cross-slot history enters for
    free through the scan initial value.
  * Matmuls (B_re, B_im, C projections) run on the PE at 1 cycle/row using
    fp16 operands (B/x) and fp16 C/u; accumulation is fp32 in PSUM.
"""
import sys, os
sys.path.insert(0, '/opt/trn_rl_repo')
import numpy as np

import concourse.bass as bass
import concourse.bacc as bacc
import concourse.tile as tile
import concourse.mybir as mybir
from concourse.bass_utils import run_bass_kernel_spmd

L, H, M = 16384, 1024, 1024
NC = 8
T = 512
S = L // (T * NC)          # 4 slots
NSL = H // 128             # 8 slices

f32 = mybir.dt.float32
f32r = mybir.dt.float32r
f16 = mybir.dt.float16
AL = mybir.AluOpType
AX = mybir.AxisListType

TABLE_DT = f16   # cos/sin/rpow tables
MM_DT = f16      # B, x, C, u matmul operand dtype

_BUILD_CACHE = {}


def _build():
    if "nc" in _BUILD_CACHE:
        return _BUILD_CACHE["nc"]
    nc = bacc.Bacc("TRN2", target_bir_lowering=False, debug=False,
                   num_devices=NC)

    xt_d = nc.dram_tensor("xt", [S, M, T], MM_DT, kind="ExternalInput").ap()
    brt_d = nc.dram_tensor("brt", [M, H], MM_DT, kind="ExternalInput").ap()
    bit_d = nc.dram_tensor("bit", [M, H], MM_DT, kind="ExternalInput").ap()
    ct_d = nc.dram_tensor("ct", [H, M], MM_DT, kind="ExternalInput").ap()
    cos_d = nc.dram_tensor("costb", [H, T], TABLE_DT, kind="ExternalInput").ap()
    sin_d = nc.dram_tensor("sintb", [H, T], TABLE_DT, kind="ExternalInput").ap()
    rpow_d = nc.dram_tensor("rpowtb", [H, T], TABLE_DT, kind="ExternalInput").ap()
    consts_d = nc.dram_tensor("consts", [128, 128], f32, kind="ExternalInput").ap()
    cwfre_d = nc.dram_tensor("cwfre", [128, 128], f32, kind="ExternalInput").ap()
    cwfim_d = nc.dram_tensor("cwfim", [128, 128], f32, kind="ExternalInput").ap()
    rwfre_d = nc.dram_tensor("rwfre", [128, 128], f32, kind="ExternalInput").ap()
    rwfim_d = nc.dram_tensor("rwfim", [128, 128], f32, kind="ExternalInput").ap()
    ident_d = nc.dram_tensor("ident", [128, 128], f32, kind="ExternalInput").ap()
    y_d = nc.dram_tensor("y", [S, M, T], f32, kind="ExternalOutput").ap()

    with tile.TileContext(nc) as tc:
        with tc.tile_pool(name="pw", bufs=1) as pw, \
             tc.tile_pool(name="px", bufs=1) as px, \
             tc.tile_pool(name="pg", bufs=1) as pg, \
             tc.tile_pool(name="pc", bufs=1) as pcp, \
             tc.tile_pool(name="pp", bufs=1, space="PSUM") as pp, \
             tc.tile_pool(name="pd", bufs=1, space="DRAM") as pd:

            # ---------- persistent weights / tables ----------
            brt_sb = []
            bit_sb = []
            ct_sb = []
            cos_sb = []
            sin_sb = []
            rpow_sb = []
            for d in range(NSL):
                t_ = pw.tile([128, H], MM_DT, name=f"brt{d}")
                nc.sync.dma_start(t_[:], brt_d[d * 128:(d + 1) * 128, :])
                brt_sb.append(t_)
                t_ = pw.tile([128, H], MM_DT, name=f"bit{d}")
                nc.sync.dma_start(t_[:], bit_d[d * 128:(d + 1) * 128, :])
                bit_sb.append(t_)
                ct_sb.append(pw.tile([128, M], MM_DT, name=f"ct{d}"))
                cos_sb.append(pw.tile([128, T], TABLE_DT, name=f"cos{d}"))
                sin_sb.append(pw.tile([128, T], TABLE_DT, name=f"sin{d}"))
                rpow_sb.append(pw.tile([128, T], TABLE_DT, name=f"rpow{d}"))

            ident_sb = pw.tile([128, 128], f32, name="ident")
            nc.sync.dma_start(ident_sb[:], ident_d)
            cwfre_sb = pw.tile([128, 128], f32, name="cwfre")
            cwfim_sb = pw.tile([128, 128], f32, name="cwfim")
            rwfre_sb = pw.tile([128, 128], f32, name="rwfre")
            rwfim_sb = pw.tile([128, 128], f32, name="rwfim")

            def emit_deferred_tables():
                for d in range(NSL):
                    nc.sync.dma_start(cos_sb[d][:], cos_d[d * 128:(d + 1) * 128, :])
                    nc.sync.dma_start(sin_sb[d][:], sin_d[d * 128:(d + 1) * 128, :])
                for d in range(NSL):
                    nc.sync.dma_start(ct_sb[d][:], ct_d[d * 128:(d + 1) * 128, :])
                    nc.sync.dma_start(rpow_sb[d][:], rpow_d[d * 128:(d + 1) * 128, :])
                nc.sync.dma_start(cwfre_sb[:], cwfre_d)
                nc.sync.dma_start(cwfim_sb[:], cwfim_d)
                nc.sync.dma_start(rwfre_sb[:], rwfre_d)
                nc.sync.dma_start(rwfim_sb[:], rwfim_d)

            # consts: DMA [128,128] then transpose once -> columns
            craw = pw.tile([128, 128], f32, name="craw")
            nc.sync.dma_start(craw[:], consts_d)
            cps = pp.tile([128, 128], f32, name="cps", tag="tp", bufs=2)
            nc.tensor.transpose(cps[:], craw[:], ident_sb[:])
            cT = pw.tile([128, 128], f32, name="cT")
            nc.vector.tensor_copy(cT[:], cps[:])

            def ccv(q, pt):
                # [128, 8] strided view of const block q, part pt (0=re,1=im)
                return cT[:, 16 * q + pt:16 * q + 16:2]

            def ccol(q, sl, pt):
                return cT[:, 16 * q + 2 * sl + pt:16 * q + 2 * sl + pt + 1]

            # persistent carry state (R only; scans are zero-init and all
            # carry history enters via the post-scan stt correction)
            zR_re = pcp.tile([128, 8], f32, name="zR_re")
            zR_im = pcp.tile([128, 8], f32, name="zR_im")
            nc.vector.memzero(zR_re[:])
            nc.vector.memzero(zR_im[:])
            state = {"R_re": zR_re, "R_im": zR_im}
            saved = {}

            def emit_scanphase(s, after_xt=None):
                xt_sb = []
                for d in range(NSL):
                    t_ = px.tile([128, T], MM_DT, name=f"xt_s{s}_d{d}",
                                 tag="xt", bufs=24)
                    nc.sync.dma_start(t_[:], xt_d[s, d * 128:(d + 1) * 128, :])
                    xt_sb.append(t_)
                if after_xt is not None:
                    after_xt()

                W_re_t = []
                W_im_t = []
                for sl in range(NSL):
                    hs = slice(sl * 128, (sl + 1) * 128)
                    ps_re = pp.tile([128, T], f32, name=f"psre{s}_{sl}",
                                    tag="bure", bufs=2)
                    ps_im = pp.tile([128, T], f32, name=f"psim{s}_{sl}",
                                    tag="buim", bufs=2)
                    for d in range(NSL):
                        nc.tensor.matmul(ps_re[:], brt_sb[d][:, hs], xt_sb[d][:],
                                         start=(d == 0), stop=(d == NSL - 1))
                    for d in range(NSL):
                        nc.tensor.matmul(ps_im[:], bit_sb[d][:, hs], xt_sb[d][:],
                                         start=(d == 0), stop=(d == NSL - 1))
                    pre16 = pg.tile([128, T], f16, name=f"pre16_{s}_{sl}", tag="pre16", bufs=3)
                    pim16 = pg.tile([128, T], f16, name=f"pim16_{s}_{sl}", tag="pim16", bufs=3)
                    nc.scalar.copy(pre16[:], ps_re[:])
                    nc.scalar.copy(pim16[:], ps_im[:])
                    t0 = pg.tile([128, T], f16, name=f"t0_{s}_{sl}", tag="t0", bufs=3)
                    t1 = pg.tile([128, T], f16, name=f"t1_{s}_{sl}", tag="t1", bufs=3)
                    t2 = pg.tile([128, T], f16, name=f"t2_{s}_{sl}", tag="t2", bufs=3)
                    t3 = pg.tile([128, T], f16, name=f"t3_{s}_{sl}", tag="t3", bufs=3)
                    gre = pg.tile([128, T], f16, name=f"gre_{s}_{sl}", tag="gre", bufs=3)
                    gim = pg.tile([128, T], f16, name=f"gim_{s}_{sl}", tag="gim", bufs=3)
                    nc.vector.tensor_tensor(t0[:], cos_sb[sl][:], pre16[:], AL.mult)
                    nc.vector.tensor_tensor(t1[:], sin_sb[sl][:], pim16[:], AL.mult)
                    nc.vector.tensor_add(gre[:], t0[:], t1[:])
                    nc.vector.tensor_tensor(t2[:], cos_sb[sl][:], pim16[:], AL.mult)
                    nc.vector.tensor_tensor(t3[:], sin_sb[sl][:], pre16[:], AL.mult)
                    nc.vector.tensor_sub(gim[:], t2[:], t3[:])
                    wre = pg.tile([128, T], f16, name=f"wre_{s}_{sl}", tag="wre", bufs=24)
                    wim = pg.tile([128, T], f16, name=f"wim_{s}_{sl}", tag="wim", bufs=24)
                    rdec = ccol(5, sl, 1).broadcast_to([128, T])
                    nc.vector.tensor_tensor_scan(wre[:], rdec, gre[:],
                                                 0.0, AL.mult, AL.add)
                    nc.vector.tensor_tensor_scan(wim[:], rdec, gim[:],
                                                 0.0, AL.mult, AL.add)
                    W_re_t.append(wre)
                    W_im_t.append(wim)

                # E = ROTT1 * W_last  (standalone chunk sum), publish + AllGather
                wlre = pcp.tile([128, 8], f32, name=f"wlre{s}", tag="wl", bufs=2)
                wlim = pcp.tile([128, 8], f32, name=f"wlim{s}", tag="wl2", bufs=2)
                for sl in range(NSL):
                    nc.scalar.copy(wlre[:, sl:sl + 1], W_re_t[sl][:, T - 1:T])
                    nc.scalar.copy(wlim[:, sl:sl + 1], W_im_t[sl][:, T - 1:T])
                epack = pcp.tile([128, 16], f32, name=f"epack{s}", tag="ep", bufs=2)
                sa = pcp.tile([128, 8], f32, name=f"sa{s}", tag="sa", bufs=2)
                sb_ = pcp.tile([128, 8], f32, name=f"sb{s}", tag="sb", bufs=2)
                sc_ = pcp.tile([128, 8], f32, name=f"sc{s}", tag="sc", bufs=2)
                sd = pcp.tile([128, 8], f32, name=f"sd{s}", tag="sd", bufs=2)
                nc.gpsimd.tensor_tensor(sa[:], ccv(2, 0), wlre[:], AL.mult)
                nc.gpsimd.tensor_tensor(sb_[:], ccv(2, 1), wlim[:], AL.mult)
                nc.gpsimd.tensor_tensor(epack[:, 0:16:2], sa[:], sb_[:], AL.subtract)
                nc.gpsimd.tensor_tensor(sc_[:], ccv(2, 0), wlim[:], AL.mult)
                nc.gpsimd.tensor_tensor(sd[:], ccv(2, 1), wlre[:], AL.mult)
                nc.gpsimd.tensor_tensor(epack[:, 1:16:2], sc_[:], sd[:], AL.add)

                pub_ps = pp.tile([16, 128], f32, name=f"pubps{s}", tag="tp", bufs=2)
                nc.tensor.transpose(pub_ps[:], epack[:], ident_sb[:])
                pub_sb = pcp.tile([16, 128], f32, name=f"pubsb{s}", tag="pub", bufs=2)
                nc.vector.tensor_copy(pub_sb[:], pub_ps[:])
                pub_dr = pd.tile([16, 128], f32, name=f"pubdr{s}", tag="pubd", bufs=2)
                nc.sync.dma_start(pub_dr[:], pub_sb[:])
                gat_dr = pd.tile([128, 128], f32, name=f"gatdr{s}", tag="gatd",
                                 bufs=2, addr_space="Shared")
                nc.gpsimd.collective_compute(
                    "AllGather", AL.bypass,
                    replica_groups=[list(range(NC))],
                    ins=[pub_dr[:].opt()],
                    outs=[gat_dr[:].opt()],
                )
                saved[s] = dict(xt_sb=xt_sb, W_re_t=W_re_t, W_im_t=W_im_t,
                                gat_dr=gat_dr)

            def emit_consume(s):
                sv = saved.pop(s)
                xt_sb = sv["xt_sb"]
                W_re_t = sv["W_re_t"]
                W_im_t = sv["W_im_t"]
                eg = pcp.tile([128, 128], f32, name=f"eg{s}", tag="eg", bufs=2)
                nc.sync.dma_start(eg[:], sv["gat_dr"][:])
                et_ps = pp.tile([128, 128], f32, name=f"etps{s}", tag="tp", bufs=2)
                nc.tensor.transpose(et_ps[:], eg[:], ident_sb[:])
                et = pcp.tile([128, 128], f32, name=f"et{s}", tag="et", bufs=2)
                nc.vector.tensor_copy(et[:], et_ps[:])

                def wsum(fold_sb, nmv, nmr1, nm):
                    tmp = pcp.tile([128, 128], f32, name=f"{nmv}{s}", tag="redt", bufs=2)
                    nc.vector.tensor_tensor(tmp[:], fold_sb[:], et[:], AL.mult)
                    red1 = pcp.tile([128, 16], f32, name=f"{nmr1}{s}", tag="red1", bufs=2)
                    nc.vector.tensor_reduce(
                        red1[:].unsqueeze(2),
                        tmp[:].rearrange("p (j x) -> p x j", j=8),
                        AX.X, AL.add)
                    out = pcp.tile([128, 8], f32, name=f"{nm}{s}", tag=nm, bufs=2)
                    nc.vector.tensor_reduce(
                        out[:].unsqueeze(2),
                        red1[:].rearrange("p (sl pt) -> p sl pt", pt=2),
                        AX.X, AL.add)
                    return out

                v_re = wsum(cwfre_sb, "tmpa", "reda", "vre")
                v_im = wsum(cwfim_sb, "tmpb", "redb", "vim")
                rp_re = wsum(rwfre_sb, "tmpc", "redc", "rpre")
                rp_im = wsum(rwfim_sb, "tmpd", "redd", "rpim")

                _sc = [0]

                def t8(a, b, op):
                    _sc[0] += 1
                    out = pcp.tile([128, 8], f32, name=f"cs{s}_{_sc[0]}",
                                   tag=f"cs{_sc[0] % 12}", bufs=2)
                    nc.gpsimd.tensor_tensor(out[:], a, b, op)
                    return out[:]

                def cmul(wre_v, wim_v, zre, zim):
                    re = t8(t8(wre_v, zre, AL.mult), t8(wim_v, zim, AL.mult),
                            AL.subtract)
                    im = t8(t8(wre_v, zim, AL.mult), t8(wim_v, zre, AL.mult),
                            AL.add)
                    return re, im

                R_re, R_im = state["R_re"], state["R_im"]
                # V_total = Vsame + Q^c * R_prev  (block 0 of consts = Q^c)
                qr_re, qr_im = cmul(ccv(0, 0), ccv(0, 1), R_re[:], R_im[:])
                vt_re = t8(v_re[:], qr_re, AL.add)
                vt_im = t8(v_im[:], qr_im, AL.add)
                # V' = ROT1 * V_total
                vp_re, vp_im = cmul(ccv(3, 0), ccv(3, 1), vt_re, vt_im)
                # R_new = Q8*R + Rpart
                q8r_re, q8r_im = cmul(ccv(4, 0), ccv(4, 1), R_re[:], R_im[:])
                rn_re = pcp.tile([128, 8], f32, name=f"rnre{s}", tag="rn", bufs=2)
                rn_im = pcp.tile([128, 8], f32, name=f"rnim{s}", tag="rn2", bufs=2)
                nc.gpsimd.tensor_tensor(rn_re[:], q8r_re, rp_re[:], AL.add)
                nc.gpsimd.tensor_tensor(rn_im[:], q8r_im, rp_im[:], AL.add)
                state["R_re"], state["R_im"] = rn_re, rn_im

                u_t = []
                for sl in range(NSL):
                    wtre = pg.tile([128, T], f16, name=f"wtre{s}_{sl}", tag="wtre", bufs=3)
                    wtim = pg.tile([128, T], f16, name=f"wtim{s}_{sl}", tag="wtim", bufs=3)
                    nc.vector.scalar_tensor_tensor(
                        wtre[:], rpow_sb[sl][:], vp_re[:, sl:sl + 1], W_re_t[sl][:],
                        AL.mult, AL.add)
                    nc.vector.scalar_tensor_tensor(
                        wtim[:], rpow_sb[sl][:], vp_im[:, sl:sl + 1], W_im_t[sl][:],
                        AL.mult, AL.add)
                    p0 = pg.tile([128, T], f16, name=f"p0_{s}_{sl}", tag="p0", bufs=3)
                    p1 = pg.tile([128, T], f16, name=f"p1_{s}_{sl}", tag="p1", bufs=3)
                    nc.vector.tensor_tensor(p0[:], cos_sb[sl][:], wtre[:], AL.mult)
                    nc.vector.tensor_tensor(p1[:], sin_sb[sl][:], wtim[:], AL.mult)
                    u = pg.tile([128, T], MM_DT, name=f"u{s}_{sl}", tag="u", bufs=12)
                    nc.vector.tensor_sub(u[:], p0[:], p1[:])
                    u_t.append(u)

                for n in range(NSL):
                    ns = slice(n * 128, (n + 1) * 128)
                    psy = pp.tile([128, T], f32, name=f"psy{s}_{n}", tag="ytile", bufs=2)
                    for sl in range(NSL):
                        nc.tensor.matmul(psy[:], ct_sb[sl][:, ns], u_t[sl][:],
                                         start=(sl == 0), stop=(sl == NSL - 1))
                    yo = pg.tile([128, T], f32, name=f"yo{s}_{n}", tag="yo", bufs=2)
                    nc.vector.scalar_tensor_tensor(yo[:], xt_sb[n][:], ccol(5, n, 0),
                                                   psy[:], AL.mult, AL.add)
                    nc.sync.dma_start(y_d[s, ns, :], yo[:])

            emit_scanphase(0, after_xt=emit_deferred_tables)
            emit_scanphase(1)
            for s in range(2, S):
                emit_consume(s - 2)
                emit_scanphase(s)
            emit_consume(S - 2)
            emit_consume(S - 1)

    nc.compile()
    _BUILD_CACHE["nc"] = nc
    return nc


def _prep(inputs, A_re, A_im, B_re, B_im, C, D):
    x = np.asarray(inputs)
    A = A_re.astype(np.float64) + 1j * A_im.astype(np.float64)
    r = np.abs(A)
    th = np.angle(A)
    k = np.arange(T)
    COS = np.cos(th[:, None] * k)
    SIN = np.sin(th[:, None] * k)
    RPOW = r[:, None] ** (k + 1)
    Q = A ** T
    ROT1 = np.exp(1j * th)
    ROTT1 = np.exp(1j * th * (T - 1))
    Q8 = Q ** 8
    RW = [Q ** (7 - j) for j in range(NC)]

    np16 = np.float16 if MM_DT == f16 else np.float32
    tb16 = np.float16 if TABLE_DT == f16 else np.float32

    brt = np.ascontiguousarray(B_re.T).astype(np16)
    bit = np.ascontiguousarray(B_im.T).astype(np16)
    ct = np.ascontiguousarray(C.T).astype(np16)
    cos_t = COS.astype(tb16)
    sin_t = SIN.astype(tb16)
    rpow_t = RPOW.astype(tb16)
    ident = np.eye(128, dtype=np.float32)

    xT = np.ascontiguousarray(x.T)  # [M, L]

    def cvec_rows(z):
        # complex [H] -> rows [16, 128] (row = 2*sl + pt)
        out = np.zeros((16, 128), np.float32)
        zr = z.real.astype(np.float32).reshape(8, 128)
        zi = z.imag.astype(np.float32).reshape(8, 128)
        out[0::2] = zr
        out[1::2] = zi
        return out

    rwf_re = np.zeros((128, 128), np.float32)
    rwf_im = np.zeros((128, 128), np.float32)
    for j in range(NC):
        w = RW[j]
        wr = w.real.astype(np.float32).reshape(8, 128)
        wi = w.imag.astype(np.float32).reshape(8, 128)
        for sl in range(8):
            rwf_re[:, 16 * j + 2 * sl + 0] = wr[sl]
            rwf_re[:, 16 * j + 2 * sl + 1] = -wi[sl]
            rwf_im[:, 16 * j + 2 * sl + 0] = wi[sl]
            rwf_im[:, 16 * j + 2 * sl + 1] = wr[sl]

    in_maps = []
    for c in range(NC):
        QPC = Q ** c
        QC1 = Q ** (c + 1)
        consts = np.zeros((128, 128), np.float32)
        consts[0:16] = cvec_rows(QPC)
        consts[16:32] = cvec_rows(QC1)
        consts[32:48] = cvec_rows(ROTT1)
        consts[48:64] = cvec_rows(ROT1)
        consts[64:80] = cvec_rows(Q8)
        # block 5: rows 80..95: row 80+2*sl = D slice, row 81+2*sl = r slice
        consts[80:96] = cvec_rows(D.astype(np.float64) + 1j * r)

        cwf_re = np.zeros((128, 128), np.float32)
        cwf_im = np.zeros((128, 128), np.float32)
        for j in range(c):
            w = Q ** (c - 1 - j)
            wr = w.real.astype(np.float32).reshape(8, 128)
            wi = w.imag.astype(np.float32).reshape(8, 128)
            for sl in range(8):
                cwf_re[:, 16 * j + 2 * sl + 0] = wr[sl]
                cwf_re[:, 16 * j + 2 * sl + 1] = -wi[sl]
                cwf_im[:, 16 * j + 2 * sl + 0] = wi[sl]
                cwf_im[:, 16 * j + 2 * sl + 1] = wr[sl]

        xt = np.zeros((S, M, T), np16)
        for s in range(S):
            m = 8 * s + c
            xt[s] = xT[:, m * T:(m + 1) * T]

        in_maps.append({
            "xt": xt, "brt": brt, "bit": bit, "ct": ct,
            "costb": cos_t, "sintb": sin_t, "rpowtb": rpow_t,
            "consts": consts,
            "cwfre": cwf_re, "cwfim": cwf_im,
            "rwfre": rwf_re, "rwfim": rwf_im,
            "ident": ident,
        })
    return in_maps


LAST_RESULTS = {}


def kernel(inputs, A_re, A_im, B_re, B_im, C, D):
    nc = _build()
    in_maps = _prep(inputs, A_re, A_im, B_re, B_im, C, D)
    trace = bool(os.environ.get("DIAG_TRACE"))
    res = run_bass_kernel_spmd(nc, in_maps, core_ids=list(range(NC)),
                               trace=trace)
    LAST_RESULTS["exec_time_ns"] = res.exec_time_ns
    LAST_RESULTS["mean_exec_time_ns"] = res.mean_exec_time_ns
    yT = np.zeros((M, L), np.float32)
    for c in range(NC):
        yc = res.results[c]["y"]
        for s in range(S):
            m = 8 * s + c
            yT[:, m * T:(m + 1) * T] = yc[s]
    return np.ascontiguousarray(yT.T)


# revision 8
# speedup vs baseline: 1.5400x; 1.0231x over previous
"""Trainium2 Bass kernel for nn_DiagRNN (diagonal complex linear RNN / LRU).

  y = Re[C @ h] + D*x,  h_t = A h_{t-1} + B x_t  (A complex-diagonal)

Strategy (8 NeuronCores, sequence-parallel):
  * Sequence of L=16384 split into 32 chunks of T=512. Chunk m is processed
    by core m%8 in "slot" m//8 (interleaved assignment) so cross-core carry
    exchange is a small per-slot AllGather that pipelines behind compute.
  * Complex scan is reduced to two REAL first-order scans per chunk via a
    rotating-frame transform: with A = r*e^{i\theta},
        W_k = e^{-i\theta k} h_{mT+k}  satisfies  W_k = r W_{k-1} + g_k,
        g_k = e^{-i\theta k} (B x)_{mT+k}.
    The real scans run on the DVE hardware scan instruction
    (tensor_tensor_scan).  Pre/post rotations are elementwise with
    host-precomputed cos/sin/r^k tables.
  * Carries: cores publish standalone chunk sums E_m, AllGather them, and
    each core folds same-slot predecessors in with one fused
    scalar_tensor_tensor per (slice, re/im); cross-slot history enters for
    free through the scan initial value.
  * Matmuls (B_re, B_im, C projections) run on the PE at 1 cycle/row using
    fp16 operands (B/x) and fp16 C/u; accumulation is fp32 in PSUM.
"""
import sys, os
sys.path.insert(0, '/opt/trn_rl_repo')
import numpy as np

import concourse.bass as bass
import concourse.bacc as bacc
import concourse.tile as tile
import concourse.mybir as mybir
from concourse.bass_utils import run_bass_kernel_spmd

L, H, M = 16384, 1024, 1024
NC = 8
T = 512
S = L // (T * NC)          # 4 slots
NSL = H // 128             # 8 slices

f32 = mybir.dt.float32
f32r = mybir.dt.float32r
f16 = mybir.dt.float16
AL = mybir.AluOpType
AX = mybir.AxisListType

TABLE_DT = f16   # cos/sin/rpow tables
MM_DT = f16      # B, x, C, u matmul operand dtype

_BUILD_CACHE = {}


def _build():
    if "nc" in _BUILD_CACHE:
        return _BUILD_CACHE["nc"]
    nc = bacc.Bacc("TRN2", target_bir_lowering=False, debug=False,
                   num_devices=NC)

    xt_d = nc.dram_tensor("xt", [S, M, T], MM_DT, kind="ExternalInput").ap()
    brt_d = nc.dram_tensor("brt", [M, H], MM_DT, kind="ExternalInput").ap()
    bit_d = nc.dram_tensor("bit", [M, H], MM_DT, kind="ExternalInput").ap()
    ct_d = nc.dram_tensor("ct", [H, M], MM_DT, kind="ExternalInput").ap()
    cos_d = nc.dram_tensor("costb", [H, T], TABLE_DT, kind="ExternalInput").ap()
    sin_d = nc.dram_tensor("sintb", [H, T], TABLE_DT, kind="ExternalInput").ap()
    rpow_d = nc.dram_tensor("rpowtb", [H, T], TABLE_DT, kind="ExternalInput").ap()
    consts_d = nc.dram_tensor("consts", [128, 128], f32, kind="ExternalInput").ap()
    cwfre_d = nc.dram_tensor("cwfre", [128, 128], f32, kind="ExternalInput").ap()
    cwfim_d = nc.dram_tensor("cwfim", [128, 128], f32, kind="ExternalInput").ap()
    rwfre_d = nc.dram_tensor("rwfre", [128, 128], f32, kind="ExternalInput").ap()
    rwfim_d = nc.dram_tensor("rwfim", [128, 128], f32, kind="ExternalInput").ap()
    ident_d = nc.dram_tensor("ident", [128, 128], f32, kind="ExternalInput").ap()
    y_d = nc.dram_tensor("y", [S, M, T], f16, kind="ExternalOutput").ap()

    with tile.TileContext(nc) as tc:
        with tc.tile_pool(name="pw", bufs=1) as pw, \
             tc.tile_pool(name="px", bufs=1) as px, \
             tc.tile_pool(name="pg", bufs=1) as pg, \
             tc.tile_pool(name="pc", bufs=1) as pcp, \
             tc.tile_pool(name="pp", bufs=1, space="PSUM") as pp, \
             tc.tile_pool(name="pd", bufs=1, space="DRAM") as pd:

            # ---------- persistent weights / tables ----------
            brt_sb = []
            bit_sb = []
            ct_sb = []
            cos_sb = []
            sin_sb = []
            rpow_sb = []
            for d in range(NSL):
                brt_sb.append(pw.tile([128, H], MM_DT, name=f"brt{d}"))
                bit_sb.append(pw.tile([128, H], MM_DT, name=f"bit{d}"))
                ct_sb.append(pw.tile([128, M], MM_DT, name=f"ct{d}"))
                cos_sb.append(pw.tile([128, T], TABLE_DT, name=f"cos{d}"))
                sin_sb.append(pw.tile([128, T], TABLE_DT, name=f"sin{d}"))
                rpow_sb.append(pw.tile([128, T], TABLE_DT, name=f"rpow{d}"))

            ident_sb = pw.tile([128, 128], f32, name="ident")
            nc.sync.dma_start(ident_sb[:], ident_d)
            cwfre_sb = pw.tile([128, 128], f32, name="cwfre")
            cwfim_sb = pw.tile([128, 128], f32, name="cwfim")
            rwfre_sb = pw.tile([128, 128], f32, name="rwfre")
            rwfim_sb = pw.tile([128, 128], f32, name="rwfim")

            def emit_deferred_tables():
                for d in range(NSL):
                    nc.sync.dma_start(brt_sb[d][:], brt_d[d * 128:(d + 1) * 128, :])
                    nc.sync.dma_start(bit_sb[d][:], bit_d[d * 128:(d + 1) * 128, :])
                for d in range(NSL):
                    nc.sync.dma_start(cos_sb[d][:], cos_d[d * 128:(d + 1) * 128, :])
                    nc.sync.dma_start(sin_sb[d][:], sin_d[d * 128:(d + 1) * 128, :])
                for d in range(NSL):
                    nc.sync.dma_start(ct_sb[d][:], ct_d[d * 128:(d + 1) * 128, :])
                    nc.sync.dma_start(rpow_sb[d][:], rpow_d[d * 128:(d + 1) * 128, :])
                nc.sync.dma_start(cwfre_sb[:], cwfre_d)
                nc.sync.dma_start(cwfim_sb[:], cwfim_d)
                nc.sync.dma_start(rwfre_sb[:], rwfre_d)
                nc.sync.dma_start(rwfim_sb[:], rwfim_d)

            # consts: DMA [128,128] then transpose once -> columns
            craw = pw.tile([128, 128], f32, name="craw")
            nc.sync.dma_start(craw[:], consts_d)
            cps = pp.tile([128, 128], f32, name="cps", tag="tp", bufs=2)
            nc.tensor.transpose(cps[:], craw[:], ident_sb[:])
            cT = pw.tile([128, 128], f32, name="cT")
            nc.vector.tensor_copy(cT[:], cps[:])

            def ccv(q, pt):
                # [128, 8] strided view of const block q, part pt (0=re,1=im)
                return cT[:, 16 * q + pt:16 * q + 16:2]

            def ccol(q, sl, pt):
                return cT[:, 16 * q + 2 * sl + pt:16 * q + 2 * sl + pt + 1]

            # persistent carry state (R only; scans are zero-init and all
            # carry history enters via the post-scan stt correction)
            zR_re = pcp.tile([128, 8], f32, name="zR_re")
            zR_im = pcp.tile([128, 8], f32, name="zR_im")
            nc.vector.memzero(zR_re[:])
            nc.vector.memzero(zR_im[:])
            state = {"R_re": zR_re, "R_im": zR_im}
            saved = {}

            def emit_scanphase(s, after_xt=None):
                xt_sb = []
                for d in range(NSL):
                    t_ = px.tile([128, T], MM_DT, name=f"xt_s{s}_d{d}",
                                 tag="xt", bufs=24)
                    nc.sync.dma_start(t_[:], xt_d[s, d * 128:(d + 1) * 128, :])
                    xt_sb.append(t_)
                if after_xt is not None:
                    after_xt()

                W_re_t = []
                W_im_t = []
                for sl in range(NSL):
                    hs = slice(sl * 128, (sl + 1) * 128)
                    ps_re = pp.tile([128, T], f32, name=f"psre{s}_{sl}",
                                    tag="bure", bufs=2)
                    ps_im = pp.tile([128, T], f32, name=f"psim{s}_{sl}",
                                    tag="buim", bufs=2)
                    for d in range(NSL):
                        nc.tensor.matmul(ps_re[:], brt_sb[d][:, hs], xt_sb[d][:],
                                         start=(d == 0), stop=(d == NSL - 1))
                    for d in range(NSL):
                        nc.tensor.matmul(ps_im[:], bit_sb[d][:, hs], xt_sb[d][:],
                                         start=(d == 0), stop=(d == NSL - 1))
                    pre16 = pg.tile([128, T], f16, name=f"pre16_{s}_{sl}", tag="pre16", bufs=3)
                    pim16 = pg.tile([128, T], f16, name=f"pim16_{s}_{sl}", tag="pim16", bufs=3)
                    nc.scalar.copy(pre16[:], ps_re[:])
                    nc.scalar.copy(pim16[:], ps_im[:])
                    t0 = pg.tile([128, T], f16, name=f"t0_{s}_{sl}", tag="t0", bufs=3)
                    t1 = pg.tile([128, T], f16, name=f"t1_{s}_{sl}", tag="t1", bufs=3)
                    t2 = pg.tile([128, T], f16, name=f"t2_{s}_{sl}", tag="t2", bufs=3)
                    t3 = pg.tile([128, T], f16, name=f"t3_{s}_{sl}", tag="t3", bufs=3)
                    gre = pg.tile([128, T], f16, name=f"gre_{s}_{sl}", tag="gre", bufs=3)
                    gim = pg.tile([128, T], f16, name=f"gim_{s}_{sl}", tag="gim", bufs=3)
                    nc.vector.tensor_tensor(t0[:], cos_sb[sl][:], pre16[:], AL.mult)
                    nc.vector.tensor_tensor(t1[:], sin_sb[sl][:], pim16[:], AL.mult)
                    nc.vector.tensor_add(gre[:], t0[:], t1[:])
                    nc.vector.tensor_tensor(t2[:], cos_sb[sl][:], pim16[:], AL.mult)
                    nc.vector.tensor_tensor(t3[:], sin_sb[sl][:], pre16[:], AL.mult)
                    nc.vector.tensor_sub(gim[:], t2[:], t3[:])
                    wre = pg.tile([128, T], f16, name=f"wre_{s}_{sl}", tag="wre", bufs=24)
                    wim = pg.tile([128, T], f16, name=f"wim_{s}_{sl}", tag="wim", bufs=24)
                    rdec = ccol(5, sl, 1).broadcast_to([128, T])
                    nc.vector.tensor_tensor_scan(wre[:], rdec, gre[:],
                                                 0.0, AL.mult, AL.add)
                    nc.vector.tensor_tensor_scan(wim[:], rdec, gim[:],
                                                 0.0, AL.mult, AL.add)
                    W_re_t.append(wre)
                    W_im_t.append(wim)

                # E = ROTT1 * W_last  (standalone chunk sum), publish + AllGather
                wlre = pcp.tile([128, 8], f32, name=f"wlre{s}", tag="wl", bufs=2)
                wlim = pcp.tile([128, 8], f32, name=f"wlim{s}", tag="wl2", bufs=2)
                for sl in range(NSL):
                    nc.scalar.copy(wlre[:, sl:sl + 1], W_re_t[sl][:, T - 1:T])
                    nc.scalar.copy(wlim[:, sl:sl + 1], W_im_t[sl][:, T - 1:T])
                epack = pcp.tile([128, 16], f32, name=f"epack{s}", tag="ep", bufs=2)
                sa = pcp.tile([128, 8], f32, name=f"sa{s}", tag="sa", bufs=2)
                sb_ = pcp.tile([128, 8], f32, name=f"sb{s}", tag="sb", bufs=2)
                sc_ = pcp.tile([128, 8], f32, name=f"sc{s}", tag="sc", bufs=2)
                sd = pcp.tile([128, 8], f32, name=f"sd{s}", tag="sd", bufs=2)
                nc.gpsimd.tensor_tensor(sa[:], ccv(2, 0), wlre[:], AL.mult)
                nc.gpsimd.tensor_tensor(sb_[:], ccv(2, 1), wlim[:], AL.mult)
                nc.gpsimd.tensor_tensor(epack[:, 0:16:2], sa[:], sb_[:], AL.subtract)
                nc.gpsimd.tensor_tensor(sc_[:], ccv(2, 0), wlim[:], AL.mult)
                nc.gpsimd.tensor_tensor(sd[:], ccv(2, 1), wlre[:], AL.mult)
                nc.gpsimd.tensor_tensor(epack[:, 1:16:2], sc_[:], sd[:], AL.add)

                pub_ps = pp.tile([16, 128], f32, name=f"pubps{s}", tag="tp", bufs=2)
                nc.tensor.transpose(pub_ps[:], epack[:], ident_sb[:])
                pub_sb = pcp.tile([16, 128], f32, name=f"pubsb{s}", tag="pub", bufs=2)
                nc.vector.tensor_copy(pub_sb[:], pub_ps[:])
                pub_dr = pd.tile([16, 128], f32, name=f"pubdr{s}", tag="pubd", bufs=2)
                nc.sync.dma_start(pub_dr[:], pub_sb[:])
                gat_dr = pd.tile([128, 128], f32, name=f"gatdr{s}", tag="gatd",
                                 bufs=2, addr_space="Shared")
                nc.gpsimd.collective_compute(
                    "AllGather", AL.bypass,
                    replica_groups=[list(range(NC))],
                    ins=[pub_dr[:].opt()],
                    outs=[gat_dr[:].opt()],
                )
                saved[s] = dict(xt_sb=xt_sb, W_re_t=W_re_t, W_im_t=W_im_t,
                                gat_dr=gat_dr)

            def emit_consume(s):
                sv = saved.pop(s)
                xt_sb = sv["xt_sb"]
                W_re_t = sv["W_re_t"]
                W_im_t = sv["W_im_t"]
                eg = pcp.tile([128, 128], f32, name=f"eg{s}", tag="eg", bufs=2)
                nc.sync.dma_start(eg[:], sv["gat_dr"][:])
                et_ps = pp.tile([128, 128], f32, name=f"etps{s}", tag="tp", bufs=2)
                nc.tensor.transpose(et_ps[:], eg[:], ident_sb[:])
                et = pcp.tile([128, 128], f32, name=f"et{s}", tag="et", bufs=2)
                nc.scalar.copy(et[:], et_ps[:])

                def wsum(fold_sb, nmv, nmr1, nm):
                    tmp = pcp.tile([128, 128], f32, name=f"{nmv}{s}", tag="redt", bufs=2)
                    nc.vector.tensor_tensor(tmp[:], fold_sb[:], et[:], AL.mult)
                    red1 = pcp.tile([128, 16], f32, name=f"{nmr1}{s}", tag="red1", bufs=2)
                    nc.vector.tensor_reduce(
                        red1[:].unsqueeze(2),
                        tmp[:].rearrange("p (j x) -> p x j", j=8),
                        AX.X, AL.add)
                    out = pcp.tile([128, 8], f32, name=f"{nm}{s}", tag=nm, bufs=2)
                    nc.vector.tensor_reduce(
                        out[:].unsqueeze(2),
                        red1[:].rearrange("p (sl pt) -> p sl pt", pt=2),
                        AX.X, AL.add)
                    return out

                v_re = wsum(cwfre_sb, "tmpa", "reda", "vre")
                v_im = wsum(cwfim_sb, "tmpb", "redb", "vim")
                rp_re = wsum(rwfre_sb, "tmpc", "redc", "rpre")
                rp_im = wsum(rwfim_sb, "tmpd", "redd", "rpim")

                _sc = [0]

                def t8(a, b, op):
                    _sc[0] += 1
                    out = pcp.tile([128, 8], f32, name=f"cs{s}_{_sc[0]}",
                                   tag=f"cs{_sc[0] % 12}", bufs=2)
                    nc.gpsimd.tensor_tensor(out[:], a, b, op)
                    return out[:]

                def cmul(wre_v, wim_v, zre, zim):
                    re = t8(t8(wre_v, zre, AL.mult), t8(wim_v, zim, AL.mult),
                            AL.subtract)
                    im = t8(t8(wre_v, zim, AL.mult), t8(wim_v, zre, AL.mult),
                            AL.add)
                    return re, im

                R_re, R_im = state["R_re"], state["R_im"]
                # V_total = Vsame + Q^c * R_prev  (block 0 of consts = Q^c)
                qr_re, qr_im = cmul(ccv(0, 0), ccv(0, 1), R_re[:], R_im[:])
                vt_re = t8(v_re[:], qr_re, AL.add)
                vt_im = t8(v_im[:], qr_im, AL.add)
                # V' = ROT1 * V_total
                vp_re, vp_im = cmul(ccv(3, 0), ccv(3, 1), vt_re, vt_im)
                # R_new = Q8*R + Rpart
                q8r_re, q8r_im = cmul(ccv(4, 0), ccv(4, 1), R_re[:], R_im[:])
                rn_re = pcp.tile([128, 8], f32, name=f"rnre{s}", tag="rn", bufs=2)
                rn_im = pcp.tile([128, 8], f32, name=f"rnim{s}", tag="rn2", bufs=2)
                nc.gpsimd.tensor_tensor(rn_re[:], q8r_re, rp_re[:], AL.add)
                nc.gpsimd.tensor_tensor(rn_im[:], q8r_im, rp_im[:], AL.add)
                state["R_re"], state["R_im"] = rn_re, rn_im

                u_t = []
                for sl in range(NSL):
                    wtre = pg.tile([128, T], f16, name=f"wtre{s}_{sl}", tag="wtre", bufs=3)
                    wtim = pg.tile([128, T], f16, name=f"wtim{s}_{sl}", tag="wtim", bufs=3)
                    nc.vector.scalar_tensor_tensor(
                        wtre[:], rpow_sb[sl][:], vp_re[:, sl:sl + 1], W_re_t[sl][:],
                        AL.mult, AL.add)
                    nc.vector.scalar_tensor_tensor(
                        wtim[:], rpow_sb[sl][:], vp_im[:, sl:sl + 1], W_im_t[sl][:],
                        AL.mult, AL.add)
                    p0 = pg.tile([128, T], f16, name=f"p0_{s}_{sl}", tag="p0", bufs=3)
                    p1 = pg.tile([128, T], f16, name=f"p1_{s}_{sl}", tag="p1", bufs=3)
                    nc.vector.tensor_tensor(p0[:], cos_sb[sl][:], wtre[:], AL.mult)
                    nc.vector.tensor_tensor(p1[:], sin_sb[sl][:], wtim[:], AL.mult)
                    u = pg.tile([128, T], MM_DT, name=f"u{s}_{sl}", tag="u", bufs=12)
                    nc.vector.tensor_sub(u[:], p0[:], p1[:])
                    u_t.append(u)

                for n in range(NSL):
                    ns = slice(n * 128, (n + 1) * 128)
                    psy = pp.tile([128, T], f32, name=f"psy{s}_{n}", tag="ytile", bufs=2)
                    for sl in range(NSL):
                        nc.tensor.matmul(psy[:], ct_sb[sl][:, ns], u_t[sl][:],
                                         start=(sl == 0), stop=(sl == NSL - 1))
                    ye = pg.tile([128, T], f16, name=f"ye{s}_{n}", tag="ye", bufs=2)
                    nc.scalar.copy(ye[:], psy[:])
                    yo = pg.tile([128, T], f16, name=f"yo{s}_{n}", tag="yo", bufs=2)
                    nc.vector.scalar_tensor_tensor(yo[:], xt_sb[n][:], ccol(5, n, 0),
                                                   ye[:], AL.mult, AL.add)
                    nc.sync.dma_start(y_d[s, ns, :], yo[:])

            emit_scanphase(0, after_xt=emit_deferred_tables)
            emit_scanphase(1)
            for s in range(2, S):
                emit_consume(s - 2)
                emit_scanphase(s)
            emit_consume(S - 2)
            emit_consume(S - 1)

    nc.compile()
    _BUILD_CACHE["nc"] = nc
    return nc


def _prep(inputs, A_re, A_im, B_re, B_im, C, D):
    x = np.asarray(inputs)
    A = A_re.astype(np.float64) + 1j * A_im.astype(np.float64)
    r = np.abs(A)
    th = np.angle(A)
    k = np.arange(T)
    COS = np.cos(th[:, None] * k)
    SIN = np.sin(th[:, None] * k)
    RPOW = r[:, None] ** (k + 1)
    Q = A ** T
    ROT1 = np.exp(1j * th)
    ROTT1 = np.exp(1j * th * (T - 1))
    Q8 = Q ** 8
    RW = [Q ** (7 - j) for j in range(NC)]

    np16 = np.float16 if MM_DT == f16 else np.float32
    tb16 = np.float16 if TABLE_DT == f16 else np.float32

    brt = np.ascontiguousarray(B_re.T).astype(np16)
    bit = np.ascontiguousarray(B_im.T).astype(np16)
    ct = np.ascontiguousarray(C.T).astype(np16)
    cos_t = COS.astype(tb16)
    sin_t = SIN.astype(tb16)
    rpow_t = RPOW.astype(tb16)
    ident = np.eye(128, dtype=np.float32)

    xT = np.ascontiguousarray(x.T)  # [M, L]

    def cvec_rows(z):
        # complex [H] -> rows [16, 128] (row = 2*sl + pt)
        out = np.zeros((16, 128), np.float32)
        zr = z.real.astype(np.float32).reshape(8, 128)
        zi = z.imag.astype(np.float32).reshape(8, 128)
        out[0::2] = zr
        out[1::2] = zi
        return out

    rwf_re = np.zeros((128, 128), np.float32)
    rwf_im = np.zeros((128, 128), np.float32)
    for j in range(NC):
        w = RW[j]
        wr = w.real.astype(np.float32).reshape(8, 128)
        wi = w.imag.astype(np.float32).reshape(8, 128)
        for sl in range(8):
            rwf_re[:, 16 * j + 2 * sl + 0] = wr[sl]
            rwf_re[:, 16 * j + 2 * sl + 1] = -wi[sl]
            rwf_im[:, 16 * j + 2 * sl + 0] = wi[sl]
            rwf_im[:, 16 * j + 2 * sl + 1] = wr[sl]

    in_maps = []
    for c in range(NC):
        QPC = Q ** c
        QC1 = Q ** (c + 1)
        consts = np.zeros((128, 128), np.float32)
        consts[0:16] = cvec_rows(QPC)
        consts[16:32] = cvec_rows(QC1)
        consts[32:48] = cvec_rows(ROTT1)
        consts[48:64] = cvec_rows(ROT1)
        consts[64:80] = cvec_rows(Q8)
        # block 5: rows 80..95: row 80+2*sl = D slice, row 81+2*sl = r slice
        consts[80:96] = cvec_rows(D.astype(np.float64) + 1j * r)

        cwf_re = np.zeros((128, 128), np.float32)
        cwf_im = np.zeros((128, 128), np.float32)
        for j in range(c):
            w = Q ** (c - 1 - j)
            wr = w.real.astype(np.float32).reshape(8, 128)
            wi = w.imag.astype(np.float32).reshape(8, 128)
            for sl in range(8):
                cwf_re[:, 16 * j + 2 * sl + 0] = wr[sl]
                cwf_re[:, 16 * j + 2 * sl + 1] = -wi[sl]
                cwf_im[:, 16 * j + 2 * sl + 0] = wi[sl]
                cwf_im[:, 16 * j + 2 * sl + 1] = wr[sl]

        xt = np.zeros((S, M, T), np16)
        for s in range(S):
            m = 8 * s + c
            xt[s] = xT[:, m * T:(m + 1) * T]

        in_maps.append({
            "xt": xt, "brt": brt, "bit": bit, "ct": ct,
            "costb": cos_t, "sintb": sin_t, "rpowtb": rpow_t,
            "consts": consts,
            "cwfre": cwf_re, "cwfim": cwf_im,
            "rwfre": rwf_re, "rwfim": rwf_im,
            "ident": ident,
        })
    return in_maps


LAST_RESULTS = {}


def kernel(inputs, A_re, A_im, B_re, B_im, C, D):
    nc = _build()
    in_maps = _prep(inputs, A_re, A_im, B_re, B_im, C, D)
    trace = bool(os.environ.get("DIAG_TRACE"))
    res = run_bass_kernel_spmd(nc, in_maps, core_ids=list(range(NC)),
                               trace=trace)
    LAST_RESULTS["exec_time_ns"] = res.exec_time_ns
    LAST_RESULTS["mean_exec_time_ns"] = res.mean_exec_time_ns
    yT = np.zeros((M, L), np.float32)
    for c in range(NC):
        yc = res.results[c]["y"].astype(np.float32)
        for s in range(S):
            m = 8 * s + c
            yT[:, m * T:(m + 1) * T] = yc[s]
    return np.ascontiguousarray(yT.T)
